# revision 57
# baseline (speedup 1.0000x reference)
"""Trainium2 Bass kernel for the non-local attention block (nn_CPP_80676665688885).

Sharding: pure data-parallel over batch - 1 sample per NeuronCore (B=8, 8 cores).
BatchNorm batch-statistics are combined with a tiny (2 KB) AllReduce.

Exact algebraic simplifications used:
  - phi/g conv biases (bp, bg) and the W-conv bias bw are dropped exactly:
    bp contributes only an n-constant to the softmax logits (cancels), bg adds
    a per-channel constant to y (cancels in BatchNorm), bw likewise.
  - theta keeps bt (it contributes a per-m logit term through phi).

Precision plan (tolerance 2e-2 rel, measured 7.3e-3):
  conv theta/phi = (W_hi + W_lo)@x_hi (x rounded to bf16 once by casting DMA)
  conv g         = W_hi@x_hi          (error cancels through BN)
  fT   = phi_hi@th_hi                 (single-bf16 logits)
  y    = gT_hi^T @ exp(fT)            (g single bf16)
  W    = (Ww_hi + Ww_lo)@y_hi         (y single bf16, post 1/s normalize)
  s broadcast via 2-pass bf16 ones-matmul, then reciprocal_approx_fast on the
  full (128, NT) layout (~18-bit accurate, 5x faster than exact reciprocal).
  BN stats (S1, S2 per channel) -> AllReduce over 8 cores -> local affine
  finalize z = wy*scale + x_hi (+shift via max identity), max over n on DVE.

Schedule: conv maxpool fused off PSUM; attention m-loop software-pipelined so
TensorE never waits on ScalarE's exp; the previous tile's normalize + W-conv
is interleaved into the next tile's m-loop; warmup AllReduce sits after the
DMA-heavy head so the barrier absorbs cross-core DMA skew.
"""

import numpy as np
from contextlib import ExitStack

import concourse.bass as bass
import concourse.bacc as bacc
import concourse.tile as tile
from concourse import mybir
from concourse.bass_utils import run_bass_kernel_spmd

F32 = mybir.dt.float32
BF16 = mybir.dt.bfloat16
AF = mybir.ActivationFunctionType
ALU = mybir.AluOpType
AX = mybir.AxisListType

B = 8
C = 256
CI = 128
N = 4096          # 64*64
M = 1024          # 32*32 after 2x2 maxpool
NT = 512          # n-tile (PSUM bank width in fp32)
NTILES = N // NT  # 8
MCH = M // 128    # 8 m-chunks
CCH = C // 128    # 2 channel chunks
XSL = 1024        # x DMA slice width
EPS = 1e-5
INV_CNT = 1.0 / (B * N)
VSPLIT = 2048     # finalize STT: vector does [0,VSPLIT), gpsimd the rest

_CACHE = {}


def _build():
    nc = bacc.Bacc("TRN2", num_devices=B)

    x_d = nc.declare_dram_parameter("x", [C, N], F32, False)
    w_hi_d = {}
    w_lo_d = {}
    for nm in ("t", "p", "g"):
        w_hi_d[nm] = nc.declare_dram_parameter(f"W{nm}Thi", [C, CI], BF16, False)
        w_lo_d[nm] = nc.declare_dram_parameter(f"W{nm}Tlo", [C, CI], BF16, False)
    wwT_hi_d = nc.declare_dram_parameter("WwThi", [CI, C], BF16, False)
    wwT_lo_d = nc.declare_dram_parameter("WwTlo", [CI, C], BF16, False)
    bt_d = nc.declare_dram_parameter("bt", [CI, 1], F32, False)
    gamma_d = nc.declare_dram_parameter("gamma", [128, CCH], F32, False)
    beta_d = nc.declare_dram_parameter("beta", [128, CCH], F32, False)
    out_d = nc.declare_dram_parameter("out", [128, CCH], F32, True)

    try:
        import ml_dtypes
        _eye = np.eye(128).astype(ml_dtypes.bfloat16)
    except ImportError:
        import jax.numpy as jnp
        _eye = np.asarray(jnp.eye(128, dtype=jnp.bfloat16))
    ident_d = nc.inline_tensor(_eye, name="ident")

    warm_in = nc.dram_tensor("warm_in", [1, 8], F32)
    warm_out = nc.dram_tensor("warm_out", [1, 8], F32, addr_space="Shared")
    warm2_in = nc.dram_tensor("warm2_in", [1, 8], F32)
    warm2_out = nc.dram_tensor("warm2_out", [1, 8], F32, addr_space="Shared")
    stats_in = nc.dram_tensor("stats_in", [128, 2 * CCH], F32)
    stats_out = nc.dram_tensor("stats_out", [B * 128, 2 * CCH], F32,
                               addr_space="Shared")

    with ExitStack() as ctx:
        tc = ctx.enter_context(tile.TileContext(nc))
        consts = ctx.enter_context(tc.tile_pool(name="consts", bufs=1))
        persist = ctx.enter_context(tc.tile_pool(name="persist", bufs=1))
        efp = ctx.enter_context(tc.tile_pool(name="efp", bufs=4))
        pl1 = ctx.enter_context(tc.tile_pool(name="pl1", bufs=2))
        small = ctx.enter_context(tc.tile_pool(name="small", bufs=4))
        yhp = ctx.enter_context(tc.tile_pool(name="yhp", bufs=2))
        trsh = ctx.enter_context(tc.tile_pool(name="trsh", bufs=2))
        ps_ft = ctx.enter_context(tc.tile_pool(name="ps_ft", bufs=2, space="PSUM"))
        ps_y = ctx.enter_context(tc.tile_pool(name="ps_y", bufs=2, space="PSUM"))
        ps_s = ctx.enter_context(tc.tile_pool(name="ps_s", bufs=1, space="PSUM"))
        ps_rb = ctx.enter_context(tc.tile_pool(name="ps_rb", bufs=1, space="PSUM"))
        ps_cv = ctx.enter_context(tc.tile_pool(name="ps_cv", bufs=2, space="PSUM"))

        # ---- phi/g weights first (small, unblock the convs) ----
        w_hi = {}
        w_lo = {}
        for nm in ("p", "g", "t"):
            w_hi[nm] = consts.tile([128, CCH, CI], BF16, name=f"w_hi_{nm}")
            if nm != "g":
                w_lo[nm] = consts.tile([128, CCH, CI], BF16, name=f"w_lo_{nm}")
            for ch in range(CCH):
                cs = slice(ch * 128, (ch + 1) * 128)
                nc.sync.dma_start(out=w_hi[nm][:, ch, :], in_=w_hi_d[nm][cs, :])
                if nm != "g":
                    nc.scalar.dma_start(out=w_lo[nm][:, ch, :],
                                        in_=w_lo_d[nm][cs, :])

        # ---- x to bf16: slices spread over 3 delivery paths (SWDGE cast,
        # sync HWDGE + vector convert, scalar HWDGE + scalar convert) in
        # consumption order ----
        x_hi = [persist.tile([128, N], BF16, tag=f"xh{ch}", name=f"x_hi{ch}")
                for ch in range(CCH)]
        x32p = ctx.enter_context(tc.tile_pool(name="x32p", bufs=3))
        paths = ["sw", "sy", "sc", "sw", "sy", "sc", "sw", "sy"]
        k = 0
        for q in range(N // XSL):
            qs = slice(q * XSL, (q + 1) * XSL)
            for ch in range(CCH):
                p = paths[k]
                k += 1
                if p == "sw":
                    nc.gpsimd.dma_start(out=x_hi[ch][:, qs],
                                        in_=x_d[ch * 128:(ch + 1) * 128, qs])
                else:
                    st = x32p.tile([128, XSL], F32, tag="x32")
                    eng = nc.sync if p == "sy" else nc.scalar
                    eng.dma_start(out=st, in_=x_d[ch * 128:(ch + 1) * 128, qs])
                    if p == "sy":
                        nc.vector.tensor_copy(out=x_hi[ch][:, qs], in_=st)
                    else:
                        nc.scalar.copy(out=x_hi[ch][:, qs], in_=st)
        ww_hi = consts.tile([128, CCH, 128], BF16)
        for ch in range(CCH):
            nc.sync.dma_start(out=ww_hi[:, ch, :], in_=wwT_hi_d[:, ch * 128:(ch + 1) * 128])
        bt_sb = consts.tile([128, 1], F32)
        nc.sync.dma_start(out=bt_sb, in_=bt_d[:, :])
        gamma_sb = consts.tile([128, CCH], F32)
        beta_sb = consts.tile([128, CCH], F32)
        nc.sync.dma_start(out=gamma_sb, in_=gamma_d[:, :])
        nc.sync.dma_start(out=beta_sb, in_=beta_d[:, :])
        ones_k = consts.tile([128, 1], BF16)
        nc.vector.memset(ones_k, 1.0)
        ones_p = consts.tile([1, 128], BF16)
        nc.vector.memset(ones_p, 1.0)
        eps_sb = consts.tile([128, 1], F32)
        nc.vector.memset(eps_sb, EPS)
        # pre-warm the ACT sqrt table so the finalize doesn't pay the load
        sqwarm = small.tile([128, 1], F32, tag="sqwarm")
        nc.scalar.activation(out=sqwarm, in_=eps_sb, func=AF.Sqrt, bias=eps_sb,
                             scale=1.0)

        # ---- conv + fused maxpool for phi and g ----
        phi_pool = persist.tile([128, M], F32, tag="phip")
        g_hi = persist.tile([128, M], BF16, tag="ghi")

        def conv_mms(ps, nm, sl):
            # g tolerates single-bf16 weights (error cancels in BN); theta/phi
            # feed the softmax logits and keep the 2-term form
            terms = (w_hi[nm],) if nm == "g" else (w_hi[nm], w_lo[nm])
            nterm = len(terms) * CCH
            k = 0
            for ch in range(CCH):
                for lhs in terms:
                    nc.tensor.matmul(ps, lhsT=lhs[:, ch, :], rhs=x_hi[ch][:, sl],
                                     start=(k == 0), stop=(k == nterm - 1))
                    k += 1

        def pool_from_psum(ps, dst, it):
            # ps covers spatial rows h in [8it, 8it+8), all 64 w columns
            stage = pl1.tile([128, NT], F32, tag="pstage")
            nc.scalar.copy(out=stage, in_=ps)
            mid = pl1.tile([128, 256], F32, tag="pool1")
            pr = stage.rearrange("p (h wp t) -> p h wp t", h=8, wp=32, t=2)
            nc.vector.tensor_tensor(
                out=mid.rearrange("p (h wp) -> p h wp", h=8),
                in0=pr[:, :, :, 0], in1=pr[:, :, :, 1], op=ALU.max)
            mr = mid.rearrange("p (hp s wp) -> p hp s wp", hp=4, s=2, wp=32)
            nc.vector.tensor_tensor(
                out=dst[:, it * 128:(it + 1) * 128].rearrange(
                    "p (hp wp) -> p hp wp", hp=4),
                in0=mr[:, :, 0, :], in1=mr[:, :, 1, :], op=ALU.max)

        for it in range(NTILES):
            sl = slice(it * NT, (it + 1) * NT)
            ps = ps_cv.tile([128, NT], F32, tag="cv")
            conv_mms(ps, "p", sl)
            pool_from_psum(ps, phi_pool, it)
        for it in range(NTILES):
            sl = slice(it * NT, (it + 1) * NT)
            ps = ps_cv.tile([128, NT], F32, tag="cv")
            conv_mms(ps, "g", sl)
            pool_from_psum(ps, g_hi, it)

        # gT chunks via bf16 tensor-engine transpose
        ident = consts.tile([128, 128], BF16)
        nc.sync.dma_start(out=ident, in_=ident_d[:, :])
        gT_hi = persist.tile([128, MCH, 128], BF16, tag="gT")
        for mc in range(MCH):
            tp = ps_cv.tile([128, 128], BF16, tag="cv")
            nc.tensor.transpose(tp, g_hi[:, mc * 128:(mc + 1) * 128], ident)
            nc.scalar.copy(out=gT_hi[:, mc, :], in_=tp)

        # phi to bf16 (single-pass fT)
        phi_hi = persist.tile([128, M], BF16, tag="phih")
        nc.vector.tensor_copy(out=phi_hi, in_=phi_pool)

        # warmup AllReduce placed after the DMA-heavy head: the barrier then
        # absorbs cross-core DMA skew, so the cores arrive at the final stats
        # AllReduce nearly aligned (the compute between is deterministic).
        warm_sb = small.tile([1, 8], F32, tag="warm")
        nc.vector.memset(warm_sb, 1.0)
        nc.sync.dma_start(out=warm_in[:, :], in_=warm_sb)
        nc.gpsimd.collective_compute(
            "AllReduce", ALU.add, replica_groups=[list(range(B))],
            ins=[warm_in[:, :]], outs=[warm_out[:, :]])

        # ---- theta conv (tile 0) ----
        th_hi = persist.tile([128, N], BF16, tag="thh")

        def theta_conv(it):
            sl = slice(it * NT, (it + 1) * NT)
            ps = ps_cv.tile([128, NT], F32, tag="cv")
            conv_mms(ps, "t", sl)
            nc.vector.tensor_scalar_add(out=th_hi[:, sl], in0=ps, scalar1=bt_sb)

        theta_conv(0)

        # ---- attention + normalize + W conv, software-pipelined over tiles ----
        wy = [persist.tile([128, N], BF16, tag=f"wy{ch}", name=f"wy{ch}")
              for ch in range(CCH)]
        s1p = persist.tile([128, CCH, NTILES], F32, tag="s1p")
        s2p = persist.tile([128, CCH, NTILES], F32, tag="s2p")

        # per-tile carried state (prev tile post-processing)
        prev = {}

        def emit_prev_rb():
            # broadcast s across partitions (2-pass bf16 ones-matmul), then
            # reciprocal on the full (128, NT) layout
            p = prev
            rbps = ps_rb.tile([128, NT], F32, tag="rb")
            nc.tensor.matmul(rbps, lhsT=ones_p, rhs=p["s_hi"], start=True, stop=False)
            nc.tensor.matmul(rbps, lhsT=ones_p, rhs=p["s_lo"], start=False, stop=True)
            rb_sb = yhp.tile([128, NT], F32, tag="rbsb")
            nc.vector.reciprocal_approx_fast(out=rb_sb, in_=rbps)
            p["rb_sb"] = rb_sb

        def emit_prev_ystt():
            p = prev
            y_t = yhp.tile([128, NT], BF16, tag="yh")
            nc.vector.scalar_tensor_tensor(
                out=y_t, in0=p["yps"], scalar=1.0, in1=p["rb_sb"],
                op0=ALU.mult, op1=ALU.mult)
            p["y_hi"] = y_t

        def emit_prev_wconv(ch):
            p = prev
            it = p["it"]
            sl = slice(it * NT, (it + 1) * NT)
            wps = ps_cv.tile([128, NT], F32, tag="cv")
            nc.tensor.matmul(wps, lhsT=ww_hi[:, ch, :], rhs=p["y_hi"],
                             start=True, stop=True)
            nc.vector.tensor_scalar(
                out=wy[ch][:, sl], in0=wps, scalar1=0.0, scalar2=None,
                op0=ALU.add, op1=ALU.add, accum_out=s1p[:, ch, it:it + 1])

        def emit_prev_s2(ch):
            p = prev
            it = p["it"]
            sl = slice(it * NT, (it + 1) * NT)
            t = trsh.tile([128, NT], BF16, tag="sqtrash")
            nc.scalar.activation(out=t, in_=wy[ch][:, sl], func=AF.Square,
                                 accum_out=s2p[:, ch, it:it + 1])

        for it in range(NTILES):
            sl = slice(it * NT, (it + 1) * NT)
            has_prev = it > 0
            if it + 1 < NTILES:
                theta_conv(it + 1)

            yps = ps_y.tile([128, NT], F32, tag="yps")
            sps = ps_s.tile([1, NT], F32, tag="sps")
            fps = [None] * MCH
            efs = [None] * MCH

            def emit_ft(mc):
                ms = slice(mc * 128, (mc + 1) * 128)
                fp = ps_ft.tile([128, NT], F32, tag="ft")
                nc.tensor.matmul(fp, lhsT=phi_hi[:, ms], rhs=th_hi[:, sl],
                                 start=True, stop=True)
                ef = efp.tile([128, NT], BF16, tag="ef")
                nc.scalar.activation(out=ef, in_=fp, func=AF.Exp)
                efs[mc] = ef

            def emit_ys(mc):
                nc.tensor.matmul(yps, lhsT=gT_hi[:, mc, :], rhs=efs[mc],
                                 start=(mc == 0), stop=(mc == MCH - 1))
                nc.tensor.matmul(sps, lhsT=ones_k, rhs=efs[mc],
                                 start=(mc == 0), stop=(mc == MCH - 1))

            emit_ft(0)
            if has_prev:
                emit_prev_rb()
            emit_ft(1)
            emit_ys(0)
            emit_ft(2)
            if has_prev:
                emit_prev_ystt()
            emit_ys(1)
            emit_ft(3)
            if has_prev:
                emit_prev_wconv(0)
            emit_ys(2)
            emit_ft(4)
            if has_prev:
                emit_prev_wconv(1)
                emit_prev_s2(0)
                emit_prev_s2(1)
            emit_ys(3)
            for mc in range(5, MCH):
                emit_ft(mc)
                emit_ys(mc - 1)
            emit_ys(MCH - 1)

            # post m-loop: s -> sbuf, bf16 hi/lo split (all on-chip)
            s_sb = small.tile([1, NT], F32, tag="s1d")
            nc.vector.tensor_copy(out=s_sb, in_=sps)
            s_hi = small.tile([1, NT], BF16, tag="sTh")
            s_lo = small.tile([1, NT], BF16, tag="sTl")
            nc.vector.tensor_copy(out=s_hi, in_=s_sb)
            nc.vector.tensor_tensor(out=s_lo, in0=s_sb, in1=s_hi, op=ALU.subtract)
            prev = {"it": it, "yps": yps, "s_hi": s_hi, "s_lo": s_lo}

        # epilogue for the last tile
        emit_prev_rb()
        emit_prev_ystt()
        emit_prev_wconv(0)
        emit_prev_wconv(1)
        emit_prev_s2(0)
        emit_prev_s2(1)

        # reload the ACT sqrt table now (the loop's Exp activations evicted
        # it); the ~1.3 us load runs during the AllReduce instead of after
        sqwarm2 = small.tile([128, 1], F32, tag="sqwarm")
        nc.scalar.activation(out=sqwarm2, in_=eps_sb, func=AF.Sqrt, bias=eps_sb,
                             scale=1.0)

        # ---- combine partials, AllReduce ----
        stats_sb = small.tile([128, 2 * CCH], F32, tag="stats")
        for ch in range(CCH):
            nc.vector.tensor_reduce(out=stats_sb[:, 2 * ch:2 * ch + 1],
                                    in_=s1p[:, ch, :], axis=AX.X, op=ALU.add)
            nc.vector.tensor_reduce(out=stats_sb[:, 2 * ch + 1:2 * ch + 2],
                                    in_=s2p[:, ch, :], axis=AX.X, op=ALU.add)
        nc.sync.dma_start(out=stats_in[:, :], in_=stats_sb)
        nc.gpsimd.collective_compute(
            "AllReduce", ALU.add, replica_groups=[list(range(B))],
            ins=[stats_in[:, :]], outs=[stats_out[0:128, :]])
        stats_g = small.tile([128, 2 * CCH], F32, tag="statsg")
        nc.sync.dma_start(out=stats_g, in_=stats_out[0:128, :])

        # ---- finalize: batched [128,2] scale/shift, then affine+max ----
        out_sb = small.tile([128, CCH], F32, tag="outsb")
        mean = small.tile([128, CCH], F32, tag="fin")
        e2 = small.tile([128, CCH], F32, tag="fin")
        m2 = small.tile([128, CCH], F32, tag="fin")
        var = small.tile([128, CCH], F32, tag="fin")
        s1v = stats_g.rearrange("p (c two) -> p c two", two=2)[:, :, 0]
        s2v = stats_g.rearrange("p (c two) -> p c two", two=2)[:, :, 1]
        nc.vector.tensor_scalar_mul(out=mean, in0=s1v, scalar1=INV_CNT)
        nc.vector.tensor_scalar_mul(out=e2, in0=s2v, scalar1=INV_CNT)
        nc.scalar.square(out=m2, in_=mean)
        nc.vector.tensor_tensor(out=var, in0=e2, in1=m2, op=ALU.subtract)
        sd = small.tile([128, CCH], F32, tag="fin")
        nc.scalar.activation(out=sd, in_=var, func=AF.Sqrt, bias=eps_sb, scale=1.0)
        inv = small.tile([128, CCH], F32, tag="fin")
        nc.vector.reciprocal(out=inv, in_=sd)
        scale = small.tile([128, CCH], F32, tag="fin")
        nc.vector.tensor_tensor(out=scale, in0=inv, in1=gamma_sb, op=ALU.mult)
        negshift = small.tile([128, CCH], F32, tag="fin")
        nc.vector.tensor_tensor(out=negshift, in0=mean, in1=scale, op=ALU.mult)
        nc.vector.tensor_tensor(out=negshift, in0=negshift, in1=beta_sb,
                                op=ALU.subtract)
        for ch in range(CCH):
            # z' = wy*scale + x_hi (in place over wy)
            nc.vector.scalar_tensor_tensor(
                out=wy[ch][:, :], in0=wy[ch][:, :], scalar=scale[:, ch:ch + 1],
                in1=x_hi[ch][:, :], op0=ALU.mult, op1=ALU.add)
            mx = small.tile([128, 1], F32, tag="fin")
            nc.vector.tensor_reduce(out=mx, in_=wy[ch][:, :], axis=AX.X,
                                    op=ALU.max)
            nc.vector.tensor_tensor(out=out_sb[:, ch:ch + 1], in0=mx,
                                    in1=negshift[:, ch:ch + 1], op=ALU.subtract)
        nc.sync.dma_start(out=out_d[:, :], in_=out_sb)

    nc.compile()
    return nc


_LAST = {}


def kernel(**inputs):
    x = np.ascontiguousarray(inputs["x"], dtype=np.float32)      # (8, 256, 64, 64)
    Wg = np.asarray(inputs["Wg"], dtype=np.float32)
    Wt = np.asarray(inputs["Wt"], dtype=np.float32)
    bt = np.asarray(inputs["bt"], dtype=np.float32)
    Wp = np.asarray(inputs["Wp"], dtype=np.float32)
    Ww = np.asarray(inputs["Ww"], dtype=np.float32)
    gamma = np.asarray(inputs["gamma"], dtype=np.float32)
    beta = np.asarray(inputs["beta"], dtype=np.float32)

    if "nc" not in _CACHE:
        _CACHE["nc"] = _build()
    nc = _CACHE["nc"]

    try:
        import ml_dtypes
        bf = ml_dtypes.bfloat16
    except ImportError:
        import jax.numpy as jnp
        bf = jnp.bfloat16

    def split(w):
        hi = np.ascontiguousarray(w.astype(bf))
        lo = np.ascontiguousarray((w - hi.astype(np.float32)).astype(bf))
        return hi, lo

    WtThi, WtTlo = split(np.ascontiguousarray(Wt.T))
    WpThi, WpTlo = split(np.ascontiguousarray(Wp.T))
    WgThi, WgTlo = split(np.ascontiguousarray(Wg.T))
    WwThi, WwTlo = split(np.ascontiguousarray(Ww.T))

    shared = {
        "WtThi": WtThi, "WtTlo": WtTlo,
        "WpThi": WpThi, "WpTlo": WpTlo,
        "WgThi": WgThi, "WgTlo": WgTlo,
        "WwThi": WwThi, "WwTlo": WwTlo,
        "bt": np.ascontiguousarray(bt.reshape(CI, 1)),
        "gamma": np.ascontiguousarray(gamma.reshape(CCH, 128).T),
        "beta": np.ascontiguousarray(beta.reshape(CCH, 128).T),
    }
    in_maps = [dict(shared, x=np.ascontiguousarray(x[b].reshape(C, N)))
               for b in range(B)]
    import os
    trace = bool(int(os.environ.get("KERNEL_TRACE", "0")))
    res = run_bass_kernel_spmd(nc, in_maps, core_ids=list(range(B)), trace=trace)
    _LAST["res"] = res
    out = np.stack([np.asarray(res.results[b]["out"]).reshape(128, CCH).T.reshape(C)
                    for b in range(B)])
    return out.reshape(B, C, 1, 1).astype(np.float32)


if __name__ == "__main__":
    pass


# revision 59
# speedup vs baseline: 1.4241x; 1.4241x over previous
"""Trainium2 Bass kernel for the non-local attention block (nn_CPP_80676665688885).

Sharding: pure data-parallel over batch - 1 sample per NeuronCore (B=8, 8 cores).
BatchNorm batch-statistics are combined with a tiny (2 KB) AllReduce.

Exact algebraic simplifications used:
  - phi/g conv biases (bp, bg) and the W-conv bias bw are dropped exactly:
    bp contributes only an n-constant to the softmax logits (cancels), bg adds
    a per-channel constant to y (cancels in BatchNorm), bw likewise.
  - theta keeps bt (it contributes a per-m logit term through phi).

Precision plan (tolerance 2e-2 rel, measured 7.3e-3):
  conv theta/phi = (W_hi + W_lo)@x_hi (x rounded to bf16 once by casting DMA)
  conv g         = W_hi@x_hi          (error cancels through BN)
  fT   = phi_hi@th_hi                 (single-bf16 logits)
  y    = gT_hi^T @ exp(fT)            (g single bf16)
  W    = (Ww_hi + Ww_lo)@y_hi         (y single bf16, post 1/s normalize)
  s broadcast via 2-pass bf16 ones-matmul, then reciprocal_approx_fast on the
  full (128, NT) layout (~18-bit accurate, 5x faster than exact reciprocal).
  BN stats (S1, S2 per channel) -> AllReduce over 8 cores -> local affine
  finalize z = wy*scale + x_hi (+shift via max identity), max over n on DVE.

Schedule: conv maxpool fused off PSUM; attention m-loop software-pipelined so
TensorE never waits on ScalarE's exp; the previous tile's normalize + W-conv
is interleaved into the next tile's m-loop; warmup AllReduce sits after the
DMA-heavy head so the barrier absorbs cross-core DMA skew.
"""

import numpy as np
from contextlib import ExitStack

import concourse.bass as bass
import concourse.bacc as bacc
import concourse.tile as tile
from concourse import mybir
from concourse.bass_utils import run_bass_kernel_spmd

F32 = mybir.dt.float32
BF16 = mybir.dt.bfloat16
AF = mybir.ActivationFunctionType
ALU = mybir.AluOpType
AX = mybir.AxisListType

B = 8
C = 256
CI = 128
N = 4096          # 64*64
M = 1024          # 32*32 after 2x2 maxpool
NT = 512          # n-tile (PSUM bank width in fp32)
NTILES = N // NT  # 8
MCH = M // 128    # 8 m-chunks
CCH = C // 128    # 2 channel chunks
XSL = 1024        # x DMA slice width
EPS = 1e-5
INV_CNT = 1.0 / (B * N)
VSPLIT = 2048     # finalize STT: vector does [0,VSPLIT), gpsimd the rest

_CACHE = {}


def _build():
    nc = bacc.Bacc("TRN2", num_devices=B)

    x_d = nc.declare_dram_parameter("x", [C, N], F32, False)
    w_hi_d = {}
    w_lo_d = {}
    for nm in ("t", "p", "g"):
        w_hi_d[nm] = nc.declare_dram_parameter(f"W{nm}Thi", [C, CI], BF16, False)
        w_lo_d[nm] = nc.declare_dram_parameter(f"W{nm}Tlo", [C, CI], BF16, False)
    wwT_hi_d = nc.declare_dram_parameter("WwThi", [CI, C], BF16, False)
    wwT_lo_d = nc.declare_dram_parameter("WwTlo", [CI, C], BF16, False)
    bt_d = nc.declare_dram_parameter("bt", [CI, 1], F32, False)
    gamma_d = nc.declare_dram_parameter("gamma", [128, CCH], F32, False)
    beta_d = nc.declare_dram_parameter("beta", [128, CCH], F32, False)
    out_d = nc.declare_dram_parameter("out", [128, CCH], F32, True)

    try:
        import ml_dtypes
        _eye = np.eye(128).astype(ml_dtypes.bfloat16)
    except ImportError:
        import jax.numpy as jnp
        _eye = np.asarray(jnp.eye(128, dtype=jnp.bfloat16))
    ident_d = nc.inline_tensor(_eye, name="ident")

    warm_in = nc.dram_tensor("warm_in", [1, 8], F32)
    warm_out = nc.dram_tensor("warm_out", [1, 8], F32, addr_space="Shared")
    warm2_in = nc.dram_tensor("warm2_in", [1, 8], F32)
    warm2_out = nc.dram_tensor("warm2_out", [1, 8], F32, addr_space="Shared")
    stats_in = nc.dram_tensor("stats_in", [128, 2 * CCH], F32)
    stats_out = nc.dram_tensor("stats_out", [B * 128, 2 * CCH], F32,
                               addr_space="Shared")

    with ExitStack() as ctx:
        tc = ctx.enter_context(tile.TileContext(nc))
        consts = ctx.enter_context(tc.tile_pool(name="consts", bufs=1))
        persist = ctx.enter_context(tc.tile_pool(name="persist", bufs=1))
        efp = ctx.enter_context(tc.tile_pool(name="efp", bufs=4))
        pl1 = ctx.enter_context(tc.tile_pool(name="pl1", bufs=2))
        small = ctx.enter_context(tc.tile_pool(name="small", bufs=4))
        yhp = ctx.enter_context(tc.tile_pool(name="yhp", bufs=2))
        trsh = ctx.enter_context(tc.tile_pool(name="trsh", bufs=2))
        ps_ft = ctx.enter_context(tc.tile_pool(name="ps_ft", bufs=2, space="PSUM"))
        ps_y = ctx.enter_context(tc.tile_pool(name="ps_y", bufs=2, space="PSUM"))
        ps_s = ctx.enter_context(tc.tile_pool(name="ps_s", bufs=1, space="PSUM"))
        ps_rb = ctx.enter_context(tc.tile_pool(name="ps_rb", bufs=1, space="PSUM"))
        ps_cv = ctx.enter_context(tc.tile_pool(name="ps_cv", bufs=2, space="PSUM"))

        # ---- phi/g weights first (small, unblock the convs) ----
        w_hi = {}
        w_lo = {}
        for nm in ("p", "g", "t"):
            w_hi[nm] = consts.tile([128, CCH, CI], BF16, name=f"w_hi_{nm}")
            if nm != "g":
                w_lo[nm] = consts.tile([128, CCH, CI], BF16, name=f"w_lo_{nm}")
            for ch in range(CCH):
                cs = slice(ch * 128, (ch + 1) * 128)
                nc.sync.dma_start(out=w_hi[nm][:, ch, :], in_=w_hi_d[nm][cs, :])
                if nm != "g":
                    nc.scalar.dma_start(out=w_lo[nm][:, ch, :],
                                        in_=w_lo_d[nm][cs, :])

        # ---- x to bf16: slices spread over 3 delivery paths (SWDGE cast,
        # sync HWDGE + vector convert, scalar HWDGE + scalar convert) in
        # consumption order ----
        x_hi = [persist.tile([128, N], BF16, tag=f"xh{ch}", name=f"x_hi{ch}")
                for ch in range(CCH)]
        x32p = ctx.enter_context(tc.tile_pool(name="x32p", bufs=3))
        paths = ["sw", "sy", "sc", "sw", "sy", "sc", "sw", "sy"]
        k = 0
        for q in range(N // XSL):
            qs = slice(q * XSL, (q + 1) * XSL)
            for ch in range(CCH):
                p = paths[k]
                k += 1
                if p == "sw":
                    nc.gpsimd.dma_start(out=x_hi[ch][:, qs],
                                        in_=x_d[ch * 128:(ch + 1) * 128, qs])
                else:
                    st = x32p.tile([128, XSL], F32, tag="x32")
                    eng = nc.sync if p == "sy" else nc.scalar
                    eng.dma_start(out=st, in_=x_d[ch * 128:(ch + 1) * 128, qs])
                    if p == "sy":
                        nc.vector.tensor_copy(out=x_hi[ch][:, qs], in_=st)
                    else:
                        nc.scalar.copy(out=x_hi[ch][:, qs], in_=st)
        ww_hi = consts.tile([128, CCH, 128], BF16)
        for ch in range(CCH):
            nc.sync.dma_start(out=ww_hi[:, ch, :], in_=wwT_hi_d[:, ch * 128:(ch + 1) * 128])
        bt_sb = consts.tile([128, 1], F32)
        nc.sync.dma_start(out=bt_sb, in_=bt_d[:, :])
        gamma_sb = consts.tile([128, CCH], F32)
        beta_sb = consts.tile([128, CCH], F32)
        nc.sync.dma_start(out=gamma_sb, in_=gamma_d[:, :])
        nc.sync.dma_start(out=beta_sb, in_=beta_d[:, :])
        ones_k = consts.tile([128, 1], BF16)
        nc.vector.memset(ones_k, 1.0)
        ones_p = consts.tile([1, 128], BF16)
        nc.vector.memset(ones_p, 1.0)
        eps_sb = consts.tile([128, 1], F32)
        nc.vector.memset(eps_sb, EPS)
        # pre-warm the ACT sqrt table so the finalize doesn't pay the load
        sqwarm = small.tile([128, 1], F32, tag="sqwarm")
        nc.scalar.activation(out=sqwarm, in_=eps_sb, func=AF.Sqrt, bias=eps_sb,
                             scale=1.0)

        # ---- conv + fused maxpool for phi and g ----
        phi_pool = persist.tile([128, M], F32, tag="phip")
        g_hi = persist.tile([128, M], BF16, tag="ghi")

        def conv_mms(ps, nm, sl):
            # g tolerates single-bf16 weights (error cancels in BN); theta/phi
            # feed the softmax logits and keep the 2-term form
            terms = (w_hi[nm],) if nm == "g" else (w_hi[nm], w_lo[nm])
            nterm = len(terms) * CCH
            k = 0
            for ch in range(CCH):
                for lhs in terms:
                    nc.tensor.matmul(ps, lhsT=lhs[:, ch, :], rhs=x_hi[ch][:, sl],
                                     start=(k == 0), stop=(k == nterm - 1))
                    k += 1

        def pool_from_psum(ps, dst, it):
            # ps covers spatial rows h in [8it, 8it+8), all 64 w columns
            stage = pl1.tile([128, NT], F32, tag="pstage")
            nc.scalar.copy(out=stage, in_=ps)
            mid = pl1.tile([128, 256], F32, tag="pool1")
            pr = stage.rearrange("p (h wp t) -> p h wp t", h=8, wp=32, t=2)
            nc.vector.tensor_tensor(
                out=mid.rearrange("p (h wp) -> p h wp", h=8),
                in0=pr[:, :, :, 0], in1=pr[:, :, :, 1], op=ALU.max)
            mr = mid.rearrange("p (hp s wp) -> p hp s wp", hp=4, s=2, wp=32)
            nc.vector.tensor_tensor(
                out=dst[:, it * 128:(it + 1) * 128].rearrange(
                    "p (hp wp) -> p hp wp", hp=4),
                in0=mr[:, :, 0, :], in1=mr[:, :, 1, :], op=ALU.max)

        for it in range(NTILES):
            sl = slice(it * NT, (it + 1) * NT)
            ps = ps_cv.tile([128, NT], F32, tag="cv")
            conv_mms(ps, "p", sl)
            pool_from_psum(ps, phi_pool, it)
        for it in range(NTILES):
            sl = slice(it * NT, (it + 1) * NT)
            ps = ps_cv.tile([128, NT], F32, tag="cv")
            conv_mms(ps, "g", sl)
            pool_from_psum(ps, g_hi, it)

        # gT chunks via bf16 tensor-engine transpose
        ident = consts.tile([128, 128], BF16)
        nc.sync.dma_start(out=ident, in_=ident_d[:, :])
        gT_hi = persist.tile([128, MCH, 128], BF16, tag="gT")
        for mc in range(MCH):
            tp = ps_cv.tile([128, 128], BF16, tag="cv")
            nc.tensor.transpose(tp, g_hi[:, mc * 128:(mc + 1) * 128], ident)
            nc.scalar.copy(out=gT_hi[:, mc, :], in_=tp)

        # phi to bf16 (single-pass fT)
        phi_hi = persist.tile([128, M], BF16, tag="phih")
        nc.vector.tensor_copy(out=phi_hi, in_=phi_pool)

        # warmup AllReduce placed after the DMA-heavy head: the barrier then
        # absorbs cross-core DMA skew, so the cores arrive at the final stats
        # AllReduce nearly aligned (the compute between is deterministic).
        warm_sb = small.tile([1, 8], F32, tag="warm")
        nc.vector.memset(warm_sb, 1.0)
        nc.sync.dma_start(out=warm_in[:, :], in_=warm_sb)
        nc.gpsimd.collective_compute(
            "AllReduce", ALU.add, replica_groups=[list(range(B))],
            ins=[warm_in[:, :]], outs=[warm_out[:, :]])

        # ---- theta conv (tile 0) ----
        th_hi = persist.tile([128, N], BF16, tag="thh")

        def theta_conv(it):
            sl = slice(it * NT, (it + 1) * NT)
            ps = ps_cv.tile([128, NT], F32, tag="cv")
            conv_mms(ps, "t", sl)
            nc.vector.tensor_scalar_add(out=th_hi[:, sl], in0=ps, scalar1=bt_sb)

        theta_conv(0)

        # ---- attention + normalize + W conv, software-pipelined over tiles ----
        wy = [persist.tile([128, N], BF16, tag=f"wy{ch}", name=f"wy{ch}")
              for ch in range(CCH)]
        s1p = persist.tile([128, CCH, NTILES], F32, tag="s1p")
        s2p = persist.tile([128, CCH, NTILES], F32, tag="s2p")

        # per-tile carried state (prev tile post-processing)
        prev = {}

        def emit_prev_rb():
            # broadcast s across partitions (2-pass bf16 ones-matmul), then
            # reciprocal on the full (128, NT) layout
            p = prev
            rbps = ps_rb.tile([128, NT], F32, tag="rb")
            nc.tensor.matmul(rbps, lhsT=ones_p, rhs=p["s_hi"], start=True, stop=True)
            rb_sb = yhp.tile([128, NT], F32, tag="rbsb")
            nc.vector.reciprocal_approx_fast(out=rb_sb, in_=rbps)
            p["rb_sb"] = rb_sb

        def emit_prev_ystt():
            p = prev
            y_t = yhp.tile([128, NT], BF16, tag="yh")
            nc.vector.scalar_tensor_tensor(
                out=y_t, in0=p["yps"], scalar=1.0, in1=p["rb_sb"],
                op0=ALU.mult, op1=ALU.mult)
            p["y_hi"] = y_t

        def emit_prev_wconv(ch):
            p = prev
            it = p["it"]
            sl = slice(it * NT, (it + 1) * NT)
            wps = ps_cv.tile([128, NT], F32, tag="cv")
            nc.tensor.matmul(wps, lhsT=ww_hi[:, ch, :], rhs=p["y_hi"],
                             start=True, stop=True)
            nc.vector.tensor_scalar(
                out=wy[ch][:, sl], in0=wps, scalar1=0.0, scalar2=None,
                op0=ALU.add, op1=ALU.add, accum_out=s1p[:, ch, it:it + 1])

        def emit_prev_s2(ch):
            p = prev
            it = p["it"]
            sl = slice(it * NT, (it + 1) * NT)
            t = trsh.tile([128, NT], BF16, tag="sqtrash")
            nc.scalar.activation(out=t, in_=wy[ch][:, sl], func=AF.Square,
                                 accum_out=s2p[:, ch, it:it + 1])

        for it in range(NTILES):
            sl = slice(it * NT, (it + 1) * NT)
            has_prev = it > 0
            if it + 1 < NTILES:
                theta_conv(it + 1)

            yps = ps_y.tile([128, NT], F32, tag="yps")
            sps = ps_s.tile([1, NT], F32, tag="sps")
            fps = [None] * MCH
            efs = [None] * MCH

            def emit_ft(mc):
                ms = slice(mc * 128, (mc + 1) * 128)
                fp = ps_ft.tile([128, NT], F32, tag="ft")
                nc.tensor.matmul(fp, lhsT=phi_hi[:, ms], rhs=th_hi[:, sl],
                                 start=True, stop=True)
                ef = efp.tile([128, NT], BF16, tag="ef")
                nc.scalar.activation(out=ef, in_=fp, func=AF.Exp)
                efs[mc] = ef

            def emit_ys(mc):
                nc.tensor.matmul(yps, lhsT=gT_hi[:, mc, :], rhs=efs[mc],
                                 start=(mc == 0), stop=(mc == MCH - 1))
                nc.tensor.matmul(sps, lhsT=ones_k, rhs=efs[mc],
                                 start=(mc == 0), stop=(mc == MCH - 1))

            emit_ft(0)
            if has_prev:
                emit_prev_rb()
            emit_ft(1)
            emit_ys(0)
            emit_ft(2)
            if has_prev:
                emit_prev_ystt()
            emit_ys(1)
            emit_ft(3)
            if has_prev:
                emit_prev_wconv(0)
            emit_ys(2)
            emit_ft(4)
            if has_prev:
                emit_prev_wconv(1)
                emit_prev_s2(0)
                emit_prev_s2(1)
            emit_ys(3)
            for mc in range(5, MCH):
                emit_ft(mc)
                emit_ys(mc - 1)
            emit_ys(MCH - 1)

            # post m-loop: s -> sbuf, bf16 hi/lo split (all on-chip)
            s_sb = small.tile([1, NT], F32, tag="s1d")
            nc.vector.tensor_copy(out=s_sb, in_=sps)
            s_hi = small.tile([1, NT], BF16, tag="sTh")
            nc.vector.tensor_copy(out=s_hi, in_=s_sb)
            prev = {"it": it, "yps": yps, "s_hi": s_hi}

        # epilogue for the last tile
        emit_prev_rb()
        emit_prev_ystt()
        emit_prev_wconv(0)
        emit_prev_wconv(1)
        emit_prev_s2(0)
        emit_prev_s2(1)

        # reload the ACT sqrt table now (the loop's Exp activations evicted
        # it); the ~1.3 us load runs during the AllReduce instead of after
        sqwarm2 = small.tile([128, 1], F32, tag="sqwarm")
        nc.scalar.activation(out=sqwarm2, in_=eps_sb, func=AF.Sqrt, bias=eps_sb,
                             scale=1.0)

        # ---- combine partials, AllReduce ----
        stats_sb = small.tile([128, 2 * CCH], F32, tag="stats")
        for ch in range(CCH):
            nc.vector.tensor_reduce(out=stats_sb[:, 2 * ch:2 * ch + 1],
                                    in_=s1p[:, ch, :], axis=AX.X, op=ALU.add)
            nc.vector.tensor_reduce(out=stats_sb[:, 2 * ch + 1:2 * ch + 2],
                                    in_=s2p[:, ch, :], axis=AX.X, op=ALU.add)
        nc.sync.dma_start(out=stats_in[:, :], in_=stats_sb)
        nc.gpsimd.collective_compute(
            "AllReduce", ALU.add, replica_groups=[list(range(B))],
            ins=[stats_in[:, :]], outs=[stats_out[0:128, :]])
        stats_g = small.tile([128, 2 * CCH], F32, tag="statsg")
        nc.sync.dma_start(out=stats_g, in_=stats_out[0:128, :])

        # ---- finalize: batched [128,2] scale/shift, then affine+max ----
        out_sb = small.tile([128, CCH], F32, tag="outsb")
        mean = small.tile([128, CCH], F32, tag="fin")
        e2 = small.tile([128, CCH], F32, tag="fin")
        m2 = small.tile([128, CCH], F32, tag="fin")
        var = small.tile([128, CCH], F32, tag="fin")
        s1v = stats_g.rearrange("p (c two) -> p c two", two=2)[:, :, 0]
        s2v = stats_g.rearrange("p (c two) -> p c two", two=2)[:, :, 1]
        nc.vector.tensor_scalar_mul(out=mean, in0=s1v, scalar1=INV_CNT)
        nc.vector.tensor_scalar_mul(out=e2, in0=s2v, scalar1=INV_CNT)
        nc.scalar.square(out=m2, in_=mean)
        nc.vector.tensor_tensor(out=var, in0=e2, in1=m2, op=ALU.subtract)
        sd = small.tile([128, CCH], F32, tag="fin")
        nc.scalar.activation(out=sd, in_=var, func=AF.Sqrt, bias=eps_sb, scale=1.0)
        inv = small.tile([128, CCH], F32, tag="fin")
        nc.vector.reciprocal(out=inv, in_=sd)
        scale = small.tile([128, CCH], F32, tag="fin")
        nc.vector.tensor_tensor(out=scale, in0=inv, in1=gamma_sb, op=ALU.mult)
        negshift = small.tile([128, CCH], F32, tag="fin")
        nc.vector.tensor_tensor(out=negshift, in0=mean, in1=scale, op=ALU.mult)
        nc.vector.tensor_tensor(out=negshift, in0=negshift, in1=beta_sb,
                                op=ALU.subtract)
        for ch in range(CCH):
            # z' = wy*scale + x_hi (in place over wy)
            nc.vector.scalar_tensor_tensor(
                out=wy[ch][:, :], in0=wy[ch][:, :], scalar=scale[:, ch:ch + 1],
                in1=x_hi[ch][:, :], op0=ALU.mult, op1=ALU.add)
            mx = small.tile([128, 1], F32, tag="fin")
            nc.vector.tensor_reduce(out=mx, in_=wy[ch][:, :], axis=AX.X,
                                    op=ALU.max)
            nc.vector.tensor_tensor(out=out_sb[:, ch:ch + 1], in0=mx,
                                    in1=negshift[:, ch:ch + 1], op=ALU.subtract)
        nc.sync.dma_start(out=out_d[:, :], in_=out_sb)

    nc.compile()
    return nc


_LAST = {}


def kernel(**inputs):
    x = np.ascontiguousarray(inputs["x"], dtype=np.float32)      # (8, 256, 64, 64)
    Wg = np.asarray(inputs["Wg"], dtype=np.float32)
    Wt = np.asarray(inputs["Wt"], dtype=np.float32)
    bt = np.asarray(inputs["bt"], dtype=np.float32)
    Wp = np.asarray(inputs["Wp"], dtype=np.float32)
    Ww = np.asarray(inputs["Ww"], dtype=np.float32)
    gamma = np.asarray(inputs["gamma"], dtype=np.float32)
    beta = np.asarray(inputs["beta"], dtype=np.float32)

    if "nc" not in _CACHE:
        _CACHE["nc"] = _build()
    nc = _CACHE["nc"]

    try:
        import ml_dtypes
        bf = ml_dtypes.bfloat16
    except ImportError:
        import jax.numpy as jnp
        bf = jnp.bfloat16

    def split(w):
        hi = np.ascontiguousarray(w.astype(bf))
        lo = np.ascontiguousarray((w - hi.astype(np.float32)).astype(bf))
        return hi, lo

    WtThi, WtTlo = split(np.ascontiguousarray(Wt.T))
    WpThi, WpTlo = split(np.ascontiguousarray(Wp.T))
    WgThi, WgTlo = split(np.ascontiguousarray(Wg.T))
    WwThi, WwTlo = split(np.ascontiguousarray(Ww.T))

    shared = {
        "WtThi": WtThi, "WtTlo": WtTlo,
        "WpThi": WpThi, "WpTlo": WpTlo,
        "WgThi": WgThi, "WgTlo": WgTlo,
        "WwThi": WwThi, "WwTlo": WwTlo,
        "bt": np.ascontiguousarray(bt.reshape(CI, 1)),
        "gamma": np.ascontiguousarray(gamma.reshape(CCH, 128).T),
        "beta": np.ascontiguousarray(beta.reshape(CCH, 128).T),
    }
    in_maps = [dict(shared, x=np.ascontiguousarray(x[b].reshape(C, N)))
               for b in range(B)]
    import os
    trace = bool(int(os.environ.get("KERNEL_TRACE", "0")))
    res = run_bass_kernel_spmd(nc, in_maps, core_ids=list(range(B)), trace=trace)
    _LAST["res"] = res
    out = np.stack([np.asarray(res.results[b]["out"]).reshape(128, CCH).T.reshape(C)
                    for b in range(B)])
    return out.reshape(B, C, 1, 1).astype(np.float32)


if __name__ == "__main__":
    pass
